# revision 1
# baseline (speedup 1.0000x reference)
"""DecoderRNN (bidirectional-GRU greedy decoder) Trainium2 kernel, 8-core SPMD.

Strategy:
  - Vocab-parallel: each core owns a 4000-row slice of w_out / b_out and
    computes its logits slice each step.
  - GRU tensor-parallel: each core computes a 128-wide slice of each gate
    (both directions); hidden state is AllGathered (transposed layout) each
    step so every core has the full h for the next step's matmuls and for
    the output projection.
  - Greedy argmax: per-core top-1 (value, index) via vector.max/max_index,
    AllGather of the 8 candidates, local combine -> next token; embedding
    row gathered from a replicated table via indirect DMA.
  - log_softmax: per-core sum(exp(logits - m_glob)) via ACT accum_out,
    AllGather of partial sums, logZ = m + ln(S); logp written per step.
  - w_out slice (2048 x 4000 fp32, transposed) is too big for SBUF: 1500
    columns stay resident, 2500 columns are re-streamed from HBM each step.

Layouts (per core k, v0 = 4000*k, hidden slice = 128*k):
  wres   [128, 16*1500]  resident w_outT: [p, c*1500+j] = w_out[v0+j, c*128+p]
  wstream[16*5*128, 500] streamed tiles in (K-chunk, s) order
  wih/whh[128, 8*768]    gate-sliced GRU weights, transposed; column order
                         per K-chunk: [f_r f_z b_r b_z | f_n b_n] (128 each)
  hT     [128, 8*64]     full hidden transposed: [p, c*64 + dir*32 + b]
  xT     [128, 8*32]     embedded token transposed: [p, c*32 + b]
  logits [128, 1000]     [32*j + b, g*500 + f] = logits[b, v0+(g*4+j)*500+f]
"""

import numpy as np

import concourse.bass as bass
import concourse.bacc as bacc
import concourse.mybir as mybir
import concourse.tile as tile
import concourse.bass_utils as bass_utils
from concourse.masks import make_identity

F32 = mybir.dt.float32
U32 = mybir.dt.uint32
AF = mybir.ActivationFunctionType
ALU = mybir.AluOpType
AX = mybir.AxisListType

B = 32
H = 1024
V = 32000
NC = 8
Vs = V // NC          # 4000 vocab rows per core
Hs = H // NC          # 128 hidden dims per core
KC = 16               # K-chunks of 128 over 2H
NCH = 8               # n-chunks of 500 over Vs
CH = 500              # n-chunk width (one PSUM bank)
RES = 3               # default resident n-chunks
STR = NCH - RES       # (per-build values passed explicitly)
GROUPS = 2            # col-tile groups of 4 chunks
BIG = 1.0e30


def build_program(T: int, debug: bool = False, res: int = RES,
                  col_tile: bool = True, fill: int = 0,
                  no_proj: bool = False, fake_stream: bool = False,
                  no_ag13: bool = False, no_ag1: bool = False,
                  no_ag2: bool = False):
    STR = NCH - res
    RES = res
    nc = bacc.Bacc("TRN2", target_bir_lowering=False, debug=False, num_devices=NC)
    dbg = {}
    if debug:
        dbg["srz"] = nc.dram_tensor("dbg_srz", [B, 512], F32, kind="ExternalOutput")
        dbg["n"] = nc.dram_tensor("dbg_n", [B, 256], F32, kind="ExternalOutput")
        dbg["hnew"] = nc.dram_tensor("dbg_hnew", [B, 256], F32, kind="ExternalOutput")
        dbg["logits"] = nc.dram_tensor("dbg_logits", [128, 1000], F32, kind="ExternalOutput")
        dbg["mg"] = nc.dram_tensor("dbg_mg", [B, 1], F32, kind="ExternalOutput")
        dbg["ig"] = nc.dram_tensor("dbg_ig", [B, 1], F32, kind="ExternalOutput")
        dbg["sg"] = nc.dram_tensor("dbg_sg", [B, 1], F32, kind="ExternalOutput")
        dbg["hT"] = nc.dram_tensor("dbg_hT", [128, 512], F32, kind="ExternalOutput")
        dbg["rzps"] = nc.dram_tensor("dbg_rzps", [B, 512], F32, kind="ExternalOutput")
        dbg["inhnps"] = nc.dram_tensor("dbg_inhnps", [B, 512], F32, kind="ExternalOutput")
        dbg["srz_pre"] = nc.dram_tensor("dbg_srz_pre", [B, 512], F32, kind="ExternalOutput")

    emb_t = nc.dram_tensor("emb_t", [V, H], F32, kind="ExternalInput")
    wres_t = nc.dram_tensor("wres_t", [128, KC * RES * CH], F32, kind="ExternalInput")
    wstream_t = nc.dram_tensor("wstream_t", [max(KC * STR, 1) * 128, CH], F32, kind="ExternalInput")
    wih_t = nc.dram_tensor("wih_t", [128, 8 * 768], F32, kind="ExternalInput")
    whh_t = nc.dram_tensor("whh_t", [128, 8 * 768], F32, kind="ExternalInput")
    brz_t = nc.dram_tensor("brz_t", [B, 512], F32, kind="ExternalInput")
    bin_t = nc.dram_tensor("bin_t", [B, 256], F32, kind="ExternalInput")
    bhn_t = nc.dram_tensor("bhn_t", [B, 256], F32, kind="ExternalInput")
    bout_t = nc.dram_tensor("bout_t", [128, GROUPS * CH], F32, kind="ExternalInput")
    offs_t = nc.dram_tensor("offs_t", [128, 1], F32, kind="ExternalInput")
    ht0_t = nc.dram_tensor("ht0_t", [128, 8 * 64], F32, kind="ExternalInput")
    hbm0_t = nc.dram_tensor("hbm0_t", [B, 256], F32, kind="ExternalInput")
    x0t_t = nc.dram_tensor("x0t_t", [128, 8 * 32], F32, kind="ExternalInput")
    logp_t = nc.dram_tensor("logp_t", [T * 128, GROUPS * CH], F32, kind="ExternalOutput")

    rg = [list(range(NC))]

    with tile.TileContext(nc) as tc:
        with (
            tc.tile_pool(name="const", bufs=1) as cpool,
            tc.tile_pool(name="stream", bufs=3) as spool,
            tc.tile_pool(name="gate", bufs=1) as gpool,
            tc.tile_pool(name="lg", bufs=2) as lpool,
            tc.tile_pool(name="stats", bufs=2) as tpool,
            tc.tile_pool(name="ps_rz", bufs=1, space="PSUM") as ps_rz_pool,
            tc.tile_pool(name="ps_n", bufs=1, space="PSUM") as ps_n_pool,
            tc.tile_pool(name="ps_proj", bufs=1, space="PSUM") as ps_proj_pool,
            tc.tile_pool(name="ps_tr", bufs=1, space="PSUM") as ps_tr_pool,
            tc.tile_pool(name="ps_exp", bufs=1, space="PSUM") as ps_exp_pool,
            tc.tile_pool(name="dram", bufs=2, space="DRAM") as dpool,
        ):
            # ---- resident loads ----
            ident = cpool.tile([128, 128], F32, name="ident")
            make_identity(nc, ident[:])
            id32 = ident[0:32, 0:32]
            wres = cpool.tile([128, KC * RES * CH], F32, name="wres")
            nc.sync.dma_start(wres[:], wres_t.ap())
            wih = cpool.tile([128, 8 * 768], F32, name="wih")
            nc.sync.dma_start(wih[:], wih_t.ap())
            whh = cpool.tile([128, 8 * 768], F32, name="whh")
            nc.sync.dma_start(whh[:], whh_t.ap())
            brz = cpool.tile([B, 512], F32, name="brz")
            nc.sync.dma_start(brz[:], brz_t.ap())
            b_in = cpool.tile([B, 256], F32, name="b_in")
            nc.sync.dma_start(b_in[:], bin_t.ap())
            b_hn = cpool.tile([B, 256], F32, name="b_hn")
            nc.sync.dma_start(b_hn[:], bhn_t.ap())
            bout = cpool.tile([128, GROUPS * CH], F32, name="bout")
            nc.sync.dma_start(bout[:], bout_t.ap())
            offs = cpool.tile([128, 1], F32, name="offs")
            nc.sync.dma_start(offs[:], offs_t.ap())
            bigt = cpool.tile([B, 8], F32, name="bigt")
            nc.vector.memset(bigt[:], BIG)

            # ping-pong state
            hT = [cpool.tile([128, 8 * 64], F32, name=f"hT{i}") for i in range(2)]
            xT = [cpool.tile([128, 8 * 32], F32, name=f"xT{i}") for i in range(2)]
            hbm = [cpool.tile([B, 256], F32, name=f"hbm{i}") for i in range(2)]
            nc.sync.dma_start(hT[0][:], ht0_t.ap())
            nc.sync.dma_start(xT[0][:], x0t_t.ap())
            nc.sync.dma_start(hbm[0][:], hbm0_t.ap())

            def emit_gh(t, rz_ps, hn_ps):
                """h-side GRU matmuls for step t (reads hT[t%2] = h(t-1))."""
                h = hT[t % 2]
                for c in range(8):
                    hf = h[:, c * 64 : c * 64 + 32]
                    hb = h[:, c * 64 + 32 : c * 64 + 64]
                    w = whh[:, c * 768 : (c + 1) * 768]
                    # start=True zeroes the whole 2KB PSUM bank: exactly one
                    # bank-clearing MM per bank per step, everything else adds.
                    nc.tensor.matmul(rz_ps[:, 0:256], lhsT=hf, rhs=w[:, 0:256],
                                     start=(c == 0), stop=False)
                    nc.tensor.matmul(rz_ps[:, 256:512], lhsT=hb, rhs=w[:, 256:512],
                                     start=False, stop=False)
                    nc.tensor.matmul(hn_ps[:, 0:128], lhsT=hf, rhs=w[:, 512:640],
                                     start=(c == 0), stop=False)
                    nc.tensor.matmul(hn_ps[:, 128:256], lhsT=hb, rhs=w[:, 640:768],
                                     start=False, stop=False)

            # step-0 h-side prologue
            rz_ps_next = ps_rz_pool.tile([B, 512], F32, name="rz_ps", tag="rz")
            inhn_ps_next = ps_n_pool.tile([B, 512], F32, name="inhn_ps", tag="inhn")
            emit_gh(0, rz_ps_next, inhn_ps_next[:, 0:256])

            for t in range(T):
                rz_ps = rz_ps_next
                inhn_ps = inhn_ps_next
                hn_ps = inhn_ps[:, 0:256]
                in_ps = inhn_ps[:, 256:512]
                x = xT[t % 2]
                h_prev = hbm[t % 2]
                h_cur = hT[(t + 1) % 2]   # written by AG1(t)

                # ---- x-side GRU matmuls ----
                for c in range(8):
                    xc = x[:, c * 32 : (c + 1) * 32]
                    w = wih[:, c * 768 : (c + 1) * 768]
                    nc.tensor.matmul(rz_ps[:], lhsT=xc, rhs=w[:, 0:512],
                                     start=False, stop=(c == 7))
                    nc.tensor.matmul(in_ps, lhsT=xc, rhs=w[:, 512:768],
                                     start=False, stop=(c == 7))

                # ---- gates (batch-major; col order [f_r f_z b_r b_z]) ----
                if debug and t == 0:
                    rzc = tpool.tile([B, 512], F32, name="rzc", tag="rzc")
                    nc.vector.tensor_copy(rzc[:], rz_ps[:])
                    nc.sync.dma_start(dbg["rzps"].ap(), rzc[:])
                    ihc = tpool.tile([B, 512], F32, name="ihc", tag="ihc")
                    nc.vector.tensor_copy(ihc[:], inhn_ps[:])
                    nc.sync.dma_start(dbg["inhnps"].ap(), ihc[:])
                s_rz = gpool.tile([B, 512], F32, name="s_rz", tag="s_rz")
                nc.vector.tensor_add(s_rz[:], rz_ps[:], brz[:])
                if debug and t == 0:
                    nc.sync.dma_start(dbg["srz_pre"].ap(), s_rz[:])
                nc.scalar.activation(s_rz[:], s_rz[:], AF.Tanh, scale=0.5)
                nc.vector.tensor_scalar(s_rz[:], s_rz[:], 0.5, 0.5,
                                        op0=ALU.mult, op1=ALU.add)
                if debug and t == 0:
                    nc.sync.dma_start(dbg["srz"].ap(), s_rz[:])
                i_n = gpool.tile([B, 256], F32, name="i_n", tag="i_n")
                nc.vector.tensor_add(i_n[:], in_ps, b_in[:])
                h_n = gpool.tile([B, 256], F32, name="h_n", tag="h_n")
                nc.vector.tensor_add(h_n[:], hn_ps, b_hn[:])
                # h_n *= r ; h_n += i_n ; n = tanh(h_n)
                nc.vector.tensor_tensor(h_n[:, 0:128], s_rz[:, 0:128],
                                        h_n[:, 0:128], op=ALU.mult)
                nc.vector.tensor_tensor(h_n[:, 128:256], s_rz[:, 256:384],
                                        h_n[:, 128:256], op=ALU.mult)
                nc.vector.tensor_add(h_n[:], h_n[:], i_n[:])
                nc.scalar.activation(h_n[:], h_n[:], AF.Tanh)
                if debug and t == 0:
                    nc.sync.dma_start(dbg["n"].ap(), h_n[:])
                # d = (h_prev - n) * z ; h_new = n + d   (d reuses i_n)
                nc.vector.tensor_sub(i_n[:], h_prev[:], h_n[:])
                nc.vector.tensor_tensor(i_n[:, 0:128], s_rz[:, 128:256],
                                        i_n[:, 0:128], op=ALU.mult)
                nc.vector.tensor_tensor(i_n[:, 128:256], s_rz[:, 384:512],
                                        i_n[:, 128:256], op=ALU.mult)
                h_new = hbm[(t + 1) % 2]
                nc.vector.tensor_add(h_new[:], h_n[:], i_n[:])
                if debug and t == 0:
                    nc.sync.dma_start(dbg["hnew"].ap(), h_new[:])

                # ---- transpose h_new, AllGather hidden ----
                tr_ps = ps_tr_pool.tile([128, 512], F32, name="tr_ps", tag="tr")
                nc.tensor.matmul(tr_ps[:, 0:32], lhsT=h_new[:, 0:128], rhs=id32,
                                 is_transpose=True, start=True, stop=False)
                nc.tensor.matmul(tr_ps[:, 32:64], lhsT=h_new[:, 128:256], rhs=id32,
                                 is_transpose=True, start=False, stop=True)
                ag1_sb = tpool.tile([128, 64], F32, name="ag1_sb", tag="ag1_sb")
                nc.vector.tensor_copy(ag1_sb[:], tr_ps[:, 0:64])
                ag1_in = dpool.tile([128, 64], F32, name="ag1_in", tag="ag1_in")
                nc.gpsimd.dma_start(ag1_in[:], ag1_sb[:])
                ag1_out = dpool.tile([128 * NC, 64], F32, name="ag1_out",
                                     addr_space="Shared", tag="ag1_out")
                if not (no_ag13 or no_ag1):
                    nc.gpsimd.collective_compute(
                        "AllGather", ALU.bypass, replica_groups=rg,
                        ins=[ag1_in.opt()], outs=[ag1_out.opt()])
                elif no_ag1:
                    nc.gpsimd.dma_start(
                        ag1_out[:].rearrange("(c p) q -> p c q", p=128),
                        ag1_in[:].rearrange("p (c q) -> p c q", c=1).to_broadcast([128, 8, 64]))
                else:
                    nc.sync.dma_start(
                        ag1_out[:].rearrange("(c p) q -> c p q", p=128)[0:1],
                        ag1_in[:].rearrange("(c p) q -> c p q", c=1))
                nc.gpsimd.dma_start(
                    h_cur[:].rearrange("p (c q) -> p c q", c=8),
                    ag1_out[:].rearrange("(c p) q -> p c q", p=128))
                if debug and t == 0:
                    nc.sync.dma_start(dbg["hT"].ap(), h_cur[:])

                # ---- output projection ----
                pj = [ps_proj_pool.tile([128, 512], F32, name=f"pj{g}", tag=f"pj{g}")
                      for g in range(GROUPS)]
                def lh_of(c):
                    if c < 8:
                        return h_cur[:, c * 64 : c * 64 + 32]
                    return h_cur[:, (c - 8) * 64 + 32 : (c - 8) * 64 + 64]

                def proj_mm(c, ch, rhs):
                    g, j = divmod(ch, 4)
                    kw = {"tile_position": (0, 32 * j)} if col_tile else {}
                    nc.tensor.matmul(
                        pj[g][32 * j : 32 * (j + 1), 0:CH], lhsT=lh_of(c),
                        rhs=rhs, start=(c == 0), stop=(c == KC - 1),
                        skip_group_check=True, **kw)

                sts = []
                if not no_proj:
                    # streamed-tile DMAs first (maximize prefetch window), then
                    # all resident MMs (dense burst warms PE), then streamed MMs
                    for c in range(KC):
                        if STR and not fake_stream:
                            st = spool.tile([128, STR * CH], F32, name="st", tag="st")
                            for s in range(STR):
                                nc.sync.dma_start(
                                    st[:, s * CH : (s + 1) * CH],
                                    wstream_t.ap()[(c * STR + s) * 128 : (c * STR + s + 1) * 128, :])
                            sts.append(st)
                        for ch in range(RES):
                            proj_mm(c, ch, wres[:, (c * RES + ch) * CH : (c * RES + ch + 1) * CH])
                    for c in range(KC):
                        for ch in range(RES, NCH):
                            if fake_stream:
                                rhs = wres[:, (c * RES + RES - 1) * CH : (c * RES + RES) * CH]
                            else:
                                rhs = sts[c][:, (ch - RES) * CH : (ch - RES + 1) * CH]
                            proj_mm(c, ch, rhs)

                if fill:
                    fill_ps = ps_exp_pool.tile([128, 512], F32, name="fill_ps", tag="exp")
                    for fi in range(fill):
                        nc.tensor.matmul(fill_ps[0:32, 0:CH],
                                         lhsT=wres[:, 0:32], rhs=wres[:, 0:CH],
                                         start=True, stop=True,
                                         skip_group_check=True)
                # ---- logits epilogue: bias, max, argmax ----
                logits = lpool.tile([128, GROUPS * CH], F32, name="logits", tag="logits")
                cand = tpool.tile([B, 4], F32, name="cand", tag="cand")
                candi = tpool.tile([B, 4], F32, name="candi", tag="candi")
                for g in range(GROUPS):
                    lg = logits[:, g * CH : (g + 1) * CH]
                    if no_proj:
                        nc.vector.tensor_copy(lg, bout[:, g * CH : (g + 1) * CH])
                    else:
                        nc.vector.tensor_add(lg, pj[g][:, 0:CH], bout[:, g * CH : (g + 1) * CH])
                if debug and t == 0:
                    nc.sync.dma_start(dbg["logits"].ap(), logits[:])
                mx8 = tpool.tile([128, 8], F32, name="mx8", tag="mx8")
                ix8 = tpool.tile([128, 8], U32, name="ix8", tag="ix8")
                nc.vector.max(out=mx8[:], in_=logits[:])
                nc.vector.max_index(out=ix8[:], in_max=mx8[:], in_values=logits[:])
                # vocab index = offs(j) + idx + (idx >= 500) * 1500
                ixf = tpool.tile([128, 1], F32, name="ixf", tag="ixf")
                nc.vector.tensor_copy(ixf[:], ix8[:, 0:1])
                gmask = tpool.tile([128, 1], F32, name="gmask", tag="gmask")
                nc.vector.tensor_scalar(gmask[:], ixf[:], float(CH), 1500.0,
                                        op0=ALU.is_ge, op1=ALU.mult)
                nc.vector.tensor_add(ixf[:], ixf[:], gmask[:])
                nc.vector.tensor_add(ixf[:], ixf[:], offs[:])
                for j in range(4):
                    nc.vector.tensor_copy(cand[:, j : j + 1],
                                          mx8[32 * j : 32 * (j + 1), 0:1])
                    nc.vector.tensor_copy(candi[:, j : j + 1],
                                          ixf[32 * j : 32 * (j + 1), 0:1])
                m_loc = tpool.tile([B, 1], F32, name="m_loc", tag="m_loc")
                nc.vector.reduce_max(m_loc[:], cand[:], axis=AX.X)
                msk = tpool.tile([B, 4], U32, name="msk", tag="msk")
                nc.vector.tensor_scalar(msk[:], cand[:], m_loc[:], None, op0=ALU.is_equal)
                isel = tpool.tile([B, 4], F32, name="isel", tag="isel")
                nc.vector.tensor_copy(isel[:], bigt[:, 0:4])
                nc.vector.copy_predicated(isel[:], msk[:], candi[:])
                i_loc = tpool.tile([B, 1], F32, name="i_loc", tag="i_loc")
                nc.vector.tensor_reduce(i_loc[:], isel[:], axis=AX.X, op=ALU.min)

                # ---- local sum-exp (vs local max) before AG2 ----
                mneg_l = tpool.tile([128, 1], F32, name="mneg_l", tag="mneg_l")
                nc.vector.tensor_scalar_mul(mneg_l[0:B, :], m_loc[:], -1.0)
                nc.vector.tensor_copy(mneg_l[B : 2 * B, :], mneg_l[0:B, :])
                nc.vector.tensor_copy(mneg_l[2 * B :, :], mneg_l[0 : 2 * B, :])
                sparts = tpool.tile([128, 2], F32, name="sparts", tag="sparts")
                for g in range(GROUPS):
                    e_ps = ps_exp_pool.tile([128, 512], F32, name="e_ps", tag="exp")
                    nc.scalar.activation(e_ps[:, 0:CH], logits[:, g * CH : (g + 1) * CH],
                                         AF.Exp, bias=mneg_l[:, 0:1],
                                         accum_out=sparts[:, g : g + 1])
                s128 = tpool.tile([128, 1], F32, name="s128", tag="s128")
                nc.vector.tensor_add(s128[:], sparts[:, 0:1], sparts[:, 1:2])
                scand = tpool.tile([B, 4], F32, name="scand", tag="scand")
                for j in range(4):
                    nc.vector.tensor_copy(scand[:, j : j + 1],
                                          s128[32 * j : 32 * (j + 1), :])
                s_loc = tpool.tile([B, 1], F32, name="s_loc", tag="s_loc")
                nc.vector.reduce_sum(s_loc[:], scand[:], axis=AX.X)

                # ---- AG2: (m, idx, s) from all cores; global argmax + logZ ----
                ag2_sb = tpool.tile([B, 3], F32, name="ag2_sb", tag="ag2_sb")
                nc.vector.tensor_copy(ag2_sb[:, 0:1], m_loc[:])
                nc.vector.tensor_copy(ag2_sb[:, 1:2], i_loc[:])
                nc.vector.tensor_copy(ag2_sb[:, 2:3], s_loc[:])
                ag2_in = dpool.tile([B, 3], F32, name="ag2_in", tag="ag2_in")
                nc.gpsimd.dma_start(ag2_in[:], ag2_sb[:])
                ag2_out = dpool.tile([B * NC, 3], F32, name="ag2_out",
                                     addr_space="Shared", tag="ag2_out")
                if not no_ag2:
                    nc.gpsimd.collective_compute(
                        "AllGather", ALU.bypass, replica_groups=rg,
                        ins=[ag2_in.opt()], outs=[ag2_out.opt()])
                else:
                    nc.gpsimd.dma_start(
                        ag2_out[:].rearrange("(r b) c -> r b c", b=B),
                        ag2_in[:].rearrange("(r b) c -> r b c", r=1).to_broadcast([NC, B, 3]))
                unp2 = tpool.tile([B, 24], F32, name="unp2", tag="unp2")
                nc.gpsimd.dma_start(
                    unp2[:].rearrange("b (r c) -> b r c", r=NC),
                    ag2_out[:].rearrange("(r b) c -> b r c", b=B))
                vals = bass.AP(unp2.tensor, unp2[:].offset,
                               [unp2[:].ap[0], [3, 8]])
                idxs = bass.AP(unp2.tensor, unp2[:].offset + 1,
                               [unp2[:].ap[0], [3, 8]])
                svals = bass.AP(unp2.tensor, unp2[:].offset + 2,
                                [unp2[:].ap[0], [3, 8]])
                m_glob = tpool.tile([B, 1], F32, name="m_glob", tag="m_glob")
                nc.vector.reduce_max(m_glob[:], vals, axis=AX.X)
                msk2 = tpool.tile([B, 8], U32, name="msk2", tag="msk2")
                nc.vector.tensor_scalar(msk2[:], vals, m_glob[:], None, op0=ALU.is_equal)
                isel2 = tpool.tile([B, 8], F32, name="isel2", tag="isel2")
                nc.vector.tensor_copy(isel2[:], bigt[:])
                nc.vector.copy_predicated(isel2[:], msk2[:], idxs)
                i_glob = tpool.tile([B, 1], F32, name="i_glob", tag="i_glob")
                nc.vector.tensor_reduce(i_glob[:], isel2[:], axis=AX.X, op=ALU.min)
                if debug and t == 0:
                    nc.sync.dma_start(dbg["mg"].ap(), m_glob[:])
                    nc.sync.dma_start(dbg["ig"].ap(), i_glob[:])
                # S_glob = sum_k s_k * exp(m_k - M); logZ = M + ln(S_glob)
                dmx = tpool.tile([B, 8], F32, name="dmx", tag="dmx")
                nc.vector.tensor_scalar(dmx[:], vals, m_glob[:], None, op0=ALU.subtract)
                nc.scalar.activation(dmx[:], dmx[:], AF.Exp)
                nc.vector.tensor_tensor(dmx[:], dmx[:], svals, op=ALU.mult)
                s_glob = tpool.tile([B, 1], F32, name="s_glob", tag="s_glob")
                nc.vector.reduce_sum(s_glob[:], dmx[:], axis=AX.X)
                if debug and t == 0:
                    nc.sync.dma_start(dbg["sg"].ap(), s_glob[:])
                lns = tpool.tile([B, 1], F32, name="lns", tag="lns")
                nc.scalar.activation(lns[:], s_glob[:], AF.Ln)
                logz = tpool.tile([128, 1], F32, name="logz", tag="logz")
                nc.vector.tensor_add(logz[0:B, :], lns[:], m_glob[:])
                nc.vector.tensor_copy(logz[B : 2 * B, :], logz[0:B, :])
                nc.vector.tensor_copy(logz[2 * B :, :], logz[0 : 2 * B, :])

                # ---- prefetch for t+1: gh matmuls, token embed, transpose ----
                if t + 1 < T:
                    rz_ps_next = ps_rz_pool.tile([B, 512], F32, name="rz_ps", tag="rz")
                    inhn_ps_next = ps_n_pool.tile([B, 512], F32, name="inhn_ps", tag="inhn")
                    emit_gh(t + 1, rz_ps_next, inhn_ps_next[:, 0:256])
                    tok = tpool.tile([B, 1], U32, name="tok", tag="tok")
                    nc.vector.tensor_copy(tok[:], i_glob[:])
                    x_sb = tpool.tile([B, H], F32, name="x_sb", tag="x_sb", bufs=1)
                    nc.gpsimd.indirect_dma_start(
                        out=x_sb[:], out_offset=None, in_=emb_t.ap(),
                        in_offset=bass.IndirectOffsetOnAxis(ap=tok[:, 0:1], axis=0))
                    xtr_ps = ps_tr_pool.tile([128, 512], F32, name="xtr_ps", tag="tr")
                    for c in range(8):
                        nc.tensor.matmul(xtr_ps[:, c * 32 : (c + 1) * 32],
                                         lhsT=x_sb[:, c * 128 : (c + 1) * 128],
                                         rhs=id32, is_transpose=True,
                                         start=(c == 0), stop=(c == 7))
                    nc.vector.tensor_copy(xT[(t + 1) % 2][:], xtr_ps[:, 0:256])

                # ---- logp = logits - logZ; write out ----
                nc.gpsimd.tensor_scalar(logits[:], logits[:], logz[:, 0:1], None,
                                        op0=ALU.subtract)
                nc.gpsimd.dma_start(logp_t.ap()[t * 128 : (t + 1) * 128, :], logits[:])

    nc.compile()
    return nc


def prep_inputs(inputs, hidden, emb, w_ih_f, w_hh_f, b_ih_f, b_hh_f,
                w_ih_b, w_hh_b, b_ih_b, b_hh_b, w_out, b_out):
    """Build the per-core input maps (all numpy, host-side sharding)."""
    emb = np.ascontiguousarray(np.asarray(emb), dtype=np.float32)
    w_out = np.asarray(w_out)
    tok0 = np.asarray(inputs)[:, 0].astype(np.int64)
    x0 = emb[tok0]                                              # (B, H)
    hidden = np.asarray(hidden)
    h_f0, h_b0 = hidden[0], hidden[1]                           # (B, H)

    x0t = np.ascontiguousarray(x0.T).reshape(8, 128, B).transpose(1, 0, 2) \
        .reshape(128, 8 * B).astype(np.float32)
    ht0 = np.empty((128, 8, 64), dtype=np.float32)
    ht0[:, :, 0:32] = np.ascontiguousarray(h_f0.T).reshape(8, 128, B).transpose(1, 0, 2)
    ht0[:, :, 32:64] = np.ascontiguousarray(h_b0.T).reshape(8, 128, B).transpose(1, 0, 2)
    ht0 = ht0.reshape(128, 8 * 64)

    wihf, whhf = np.asarray(w_ih_f), np.asarray(w_hh_f)
    wihb, whhb = np.asarray(w_ih_b), np.asarray(w_hh_b)
    bihf, bhhf = np.asarray(b_ih_f), np.asarray(b_hh_f)
    bihb, bhhb = np.asarray(b_ih_b), np.asarray(b_hh_b)

    in_maps = []
    for k in range(NC):
        v0 = Vs * k
        sl = [slice(g * H + Hs * k, g * H + Hs * (k + 1)) for g in range(3)]

        w_oT = np.ascontiguousarray(w_out[v0 : v0 + Vs, :].T)   # (2048, Vs)
        wres = w_oT.reshape(KC, 128, Vs)[:, :, : RES * CH] \
            .transpose(1, 0, 2).reshape(128, KC * RES * CH).astype(np.float32).copy()
        wstr = w_oT.reshape(KC, 128, NCH, CH)[:, :, RES:, :] \
            .transpose(0, 2, 1, 3).reshape(KC * STR * 128, CH).astype(np.float32).copy()

        def gcat(wf, wb):
            cols = [wf[sl[0]].T, wf[sl[1]].T, wb[sl[0]].T, wb[sl[1]].T,
                    wf[sl[2]].T, wb[sl[2]].T]
            cat = np.concatenate(cols, axis=1)                   # (1024, 768)
            return cat.reshape(8, 128, 768).transpose(1, 0, 2) \
                .reshape(128, 8 * 768).astype(np.float32).copy()

        def bcast(v):
            return np.broadcast_to(v.astype(np.float32), (B, v.size)).copy()

        brz = bcast(np.concatenate([bihf[sl[0]] + bhhf[sl[0]],
                                    bihf[sl[1]] + bhhf[sl[1]],
                                    bihb[sl[0]] + bhhb[sl[0]],
                                    bihb[sl[1]] + bhhb[sl[1]]]))
        b_in_ = bcast(np.concatenate([bihf[sl[2]], bihb[sl[2]]]))
        b_hn_ = bcast(np.concatenate([bhhf[sl[2]], bhhb[sl[2]]]))

        bo = np.asarray(b_out)[v0 : v0 + Vs].reshape(GROUPS, 4, CH)
        boutt = np.empty((128, GROUPS * CH), dtype=np.float32)
        for g in range(GROUPS):
            for j in range(4):
                boutt[32 * j : 32 * (j + 1), g * CH : (g + 1) * CH] = bo[g, j]

        # per-partition (32j+b) vocab base: v0 + j*500
        of = np.empty((128, 1), dtype=np.float32)
        for j in range(4):
            of[32 * j : 32 * (j + 1), 0] = v0 + j * CH

        hbm0 = np.concatenate([h_f0[:, Hs * k : Hs * (k + 1)],
                               h_b0[:, Hs * k : Hs * (k + 1)]], axis=1) \
            .astype(np.float32).copy()

        in_maps.append({
            "emb_t": emb, "wres_t": wres, "wstream_t": wstr,
            "wih_t": gcat(wihf, wihb), "whh_t": gcat(whhf, whhb),
            "brz_t": brz, "bin_t": b_in_, "bhn_t": b_hn_,
            "bout_t": boutt, "offs_t": of,
            "ht0_t": ht0, "hbm0_t": hbm0, "x0t_t": x0t,
        })
    return in_maps


_CACHE = {}


def _get_program(T, **kw):
    key = (T, tuple(sorted(kw.items())))
    if key not in _CACHE:
        _CACHE[key] = build_program(T, **kw)
    return _CACHE[key]


def run(T, in_maps, trace=False):
    nc = _get_program(T)
    res = bass_utils.run_bass_kernel_spmd(
        nc, in_maps, core_ids=list(range(NC)), trace=trace)
    outs = []
    for k in range(NC):
        arr = res.results[k]["logp_t"].reshape(T, 4, B, GROUPS, CH)
        outs.append(arr.transpose(2, 0, 3, 1, 4).reshape(B, T, Vs))
    return np.concatenate(outs, axis=2), res


def kernel(inputs, hidden, emb, w_ih_f, w_hh_f, b_ih_f, b_hh_f,
           w_ih_b, w_hh_b, b_ih_b, b_hh_b, w_out, b_out, output_len):
    T = int(output_len)
    in_maps = prep_inputs(inputs, hidden, emb, w_ih_f, w_hh_f, b_ih_f, b_hh_f,
                          w_ih_b, w_hh_b, b_ih_b, b_hh_b, w_out, b_out)
    out, _ = run(T, in_maps)
    return out



# revision 14
# speedup vs baseline: 1.4356x; 1.4356x over previous
"""DecoderRNN (bidirectional-GRU greedy decoder) Trainium2 kernel, 8-core SPMD.

v2 strategy (bf16-resident output projection + fp32 top-4 refinement):
  - Vocab-parallel: each core owns a 4000-row slice of w_out, resident in
    SBUF as bf16 [128, 16*8*500] -> no per-step HBM streaming and 1 cyc/col
    PE streaming (vs 4 for fp32).
  - GRU tensor-parallel over H (fp32, batch-major): each core computes a
    128-wide slice of each gate (both dirs); hidden state AllGathered
    (transposed layout) each step.
  - Greedy argmax exactness: bf16 logits carry ~1.3e-3 noise, so per step
    each core extracts top-8 approx candidates per partition-row (vector.max
    / max_index), merges to top-4 per batch row, gathers those w_out rows in
    fp32 from HBM, recomputes exact fp32 logits on the PE (transpose +
    16-chunk GEMM against the fp32 hidden), and the cross-core argmax (AG2)
    compares the refined fp32 values. Output logp keeps the bf16-accuracy
    logits (abs err ~1e-3 << tolerance).
  - log_softmax: per-core sum(exp(logits - m_approx)) via ACT accum_out;
    AG2 carries (m_refined, idx, m_approx, s); logZ = max(m_approx) +
    ln(sum s_k exp(m_approx_k - max)).

Layouts (per core k, v0 = 4000*k, hidden slice = 128*k):
  wres   [128, 16*8*500] bf16 w_outT: [p, (c*8+ch)*500+j] = w_out[v0+ch*500+j', c*128+p]
  wih/whh[128, 8*768]    gate-sliced GRU weights, transposed; column order
                         per K-chunk: [f_r f_z b_r b_z | f_n b_n] (128 each)
  hT     [128, 8*64]     full hidden transposed: [p, c*64 + dir*32 + b]
  xT     [128, 8*32]     embedded token transposed: [p, c*32 + b]
  logits [128, 1000]     [32*j + b, g*500 + f] = logits[b, v0+(g*4+j)*500+f]
"""

import numpy as np

import concourse.bass as bass
import concourse.bacc as bacc
import concourse.mybir as mybir
import concourse.tile as tile
import concourse.bass_utils as bass_utils
from concourse.masks import make_identity

F32 = mybir.dt.float32
BF16 = mybir.dt.bfloat16
U32 = mybir.dt.uint32
AF = mybir.ActivationFunctionType
ALU = mybir.AluOpType
AX = mybir.AxisListType

B = 32
H = 1024
V = 32000
NC = 8
Vs = V // NC          # 4000 vocab rows per core
Hs = H // NC          # 128 hidden dims per core
KC = 16               # K-chunks of 128 over 2H
NCH = 8               # n-chunks of 500 over Vs
CH = 500              # n-chunk width (one PSUM bank)
GROUPS = 2            # col-tile groups of 4 chunks
NREF = 4              # refined candidates per batch row
BIG = 1.0e30


def build_program(T: int):
    nc = bacc.Bacc("TRN2", target_bir_lowering=False, debug=False, num_devices=NC)

    emb_t = nc.dram_tensor("emb_t", [V, H], F32, kind="ExternalInput")
    wres_t = nc.dram_tensor("wres_t", [128, KC * NCH * CH], BF16, kind="ExternalInput")
    wsl_t = [nc.dram_tensor(f"wsl{h}_t", [Vs, 512], F32, kind="ExternalInput")
             for h in range(4)]
    boutf_t = nc.dram_tensor("boutf_t", [Vs, 1], F32, kind="ExternalInput")
    wih_t = nc.dram_tensor("wih_t", [128, 8 * 768], F32, kind="ExternalInput")
    whh_t = nc.dram_tensor("whh_t", [128, 8 * 768], F32, kind="ExternalInput")
    brz_t = nc.dram_tensor("brz_t", [B, 512], F32, kind="ExternalInput")
    bin_t = nc.dram_tensor("bin_t", [B, 256], F32, kind="ExternalInput")
    bhn_t = nc.dram_tensor("bhn_t", [B, 256], F32, kind="ExternalInput")
    bout_t = nc.dram_tensor("bout_t", [128, GROUPS * CH], BF16, kind="ExternalInput")
    offsl_t = nc.dram_tensor("offsl_t", [128, 1], F32, kind="ExternalInput")
    v0_t = nc.dram_tensor("v0_t", [B, 1], F32, kind="ExternalInput")
    onehot_t = nc.dram_tensor("onehot_t", [128, B], F32, kind="ExternalInput")
    ht0_t = nc.dram_tensor("ht0_t", [128, 8 * 64], F32, kind="ExternalInput")
    hbm0_t = nc.dram_tensor("hbm0_t", [B, 256], F32, kind="ExternalInput")
    x0t_t = nc.dram_tensor("x0t_t", [128, 8 * 32], F32, kind="ExternalInput")
    logp_t = nc.dram_tensor("logp_t", [T * 128, GROUPS * CH], F32, kind="ExternalOutput")

    rg = [list(range(NC))]

    with tile.TileContext(nc) as tc:
        with (
            tc.tile_pool(name="const", bufs=1) as cpool,
            tc.tile_pool(name="gate", bufs=1) as gpool,
            tc.tile_pool(name="lg", bufs=1) as lpool,
            tc.tile_pool(name="ref", bufs=1) as rpool,
            tc.tile_pool(name="stats", bufs=1) as tpool,
            tc.tile_pool(name="ps_rz", bufs=1, space="PSUM") as ps_rz_pool,
            tc.tile_pool(name="ps_n", bufs=1, space="PSUM") as ps_n_pool,
            tc.tile_pool(name="ps_proj", bufs=1, space="PSUM") as ps_proj_pool,
            tc.tile_pool(name="ps_tr", bufs=2, space="PSUM") as ps_tr_pool,
            tc.tile_pool(name="ps_exp", bufs=1, space="PSUM") as ps_exp_pool,
            tc.tile_pool(name="ps_ref", bufs=1, space="PSUM") as ps_ref_pool,
            tc.tile_pool(name="dram", bufs=2, space="DRAM") as dpool,
        ):
            # ---- resident loads ----
            ident = cpool.tile([128, 128], F32, name="ident")
            make_identity(nc, ident[:])
            id32 = ident[0:32, 0:32]
            wres = cpool.tile([128, KC * NCH * CH], BF16, name="wres")
            nc.sync.dma_start(wres[:], wres_t.ap())
            wih = cpool.tile([128, 8 * 768], F32, name="wih")
            nc.sync.dma_start(wih[:], wih_t.ap())
            whh = cpool.tile([128, 8 * 768], F32, name="whh")
            nc.sync.dma_start(whh[:], whh_t.ap())
            brz = cpool.tile([B, 512], F32, name="brz")
            nc.sync.dma_start(brz[:], brz_t.ap())
            b_in = cpool.tile([B, 256], F32, name="b_in")
            nc.sync.dma_start(b_in[:], bin_t.ap())
            b_hn = cpool.tile([B, 256], F32, name="b_hn")
            nc.sync.dma_start(b_hn[:], bhn_t.ap())
            bout = cpool.tile([128, GROUPS * CH], BF16, name="bout")
            nc.sync.dma_start(bout[:], bout_t.ap())
            offsl = cpool.tile([128, 1], F32, name="offsl")
            nc.sync.dma_start(offsl[:], offsl_t.ap())
            v0t = cpool.tile([B, 1], F32, name="v0t")
            nc.sync.dma_start(v0t[:], v0_t.ap())
            onehot = cpool.tile([128, B], F32, name="onehot")
            nc.sync.dma_start(onehot[:], onehot_t.ap())
            bigt = cpool.tile([B, 16], F32, name="bigt")
            nc.vector.memset(bigt[:], BIG)

            # ping-pong state
            hT = [cpool.tile([128, 8 * 64], F32, name=f"hT{i}") for i in range(2)]
            xT = [cpool.tile([128, 8 * 32], F32, name=f"xT{i}") for i in range(2)]
            hbm = [cpool.tile([B, 256], F32, name=f"hbm{i}") for i in range(2)]
            nc.sync.dma_start(hT[0][:], ht0_t.ap())
            nc.sync.dma_start(xT[0][:], x0t_t.ap())
            nc.sync.dma_start(hbm[0][:], hbm0_t.ap())

            def emit_gh(t, rz_ps, hn_ps):
                """h-side GRU matmuls for step t (reads hT[t%2] = h(t-1))."""
                h = hT[t % 2]
                for c in range(8):
                    hf = h[:, c * 64 : c * 64 + 32]
                    hb = h[:, c * 64 + 32 : c * 64 + 64]
                    w = whh[:, c * 768 : (c + 1) * 768]
                    nc.tensor.matmul(rz_ps[:, 0:256], lhsT=hf, rhs=w[:, 0:256],
                                     start=(c == 0), stop=False)
                    nc.tensor.matmul(rz_ps[:, 256:512], lhsT=hb, rhs=w[:, 256:512],
                                     start=False, stop=False)
                    nc.tensor.matmul(hn_ps[:, 0:128], lhsT=hf, rhs=w[:, 512:640],
                                     start=(c == 0), stop=False)
                    nc.tensor.matmul(hn_ps[:, 128:256], lhsT=hb, rhs=w[:, 640:768],
                                     start=False, stop=False)

            # step-0 h-side prologue
            rz_ps_next = ps_rz_pool.tile([B, 512], F32, name="rz_ps", tag="rz")
            inhn_ps_next = ps_n_pool.tile([B, 512], F32, name="inhn_ps", tag="inhn")
            emit_gh(0, rz_ps_next, inhn_ps_next[:, 0:256])

            for t in range(T):
                rz_ps = rz_ps_next
                inhn_ps = inhn_ps_next
                hn_ps = inhn_ps[:, 0:256]
                in_ps = inhn_ps[:, 256:512]
                x = xT[t % 2]
                h_prev = hbm[t % 2]
                h_cur = hT[(t + 1) % 2]   # written by AG1(t)

                # ---- x-side GRU matmuls ----
                for c in range(8):
                    xc = x[:, c * 32 : (c + 1) * 32]
                    w = wih[:, c * 768 : (c + 1) * 768]
                    nc.tensor.matmul(rz_ps[:], lhsT=xc, rhs=w[:, 0:512],
                                     start=False, stop=(c == 7))
                    nc.tensor.matmul(in_ps, lhsT=xc, rhs=w[:, 512:768],
                                     start=False, stop=(c == 7))

                # ---- gates (batch-major; col order [f_r f_z b_r b_z]) ----
                s_rz = gpool.tile([B, 512], F32, name="s_rz", tag="s_rz")
                nc.vector.tensor_add(s_rz[:], rz_ps[:], brz[:])
                nc.scalar.activation(s_rz[:], s_rz[:], AF.Tanh, scale=0.5)
                nc.vector.tensor_scalar(s_rz[:], s_rz[:], 0.5, 0.5,
                                        op0=ALU.mult, op1=ALU.add)
                i_n = gpool.tile([B, 256], F32, name="i_n", tag="i_n")
                nc.vector.tensor_add(i_n[:], in_ps, b_in[:])
                h_n = gpool.tile([B, 256], F32, name="h_n", tag="h_n")
                nc.vector.tensor_add(h_n[:], hn_ps, b_hn[:])
                # h_n *= r ; h_n += i_n ; n = tanh(h_n)
                nc.vector.tensor_tensor(h_n[:, 0:128], s_rz[:, 0:128],
                                        h_n[:, 0:128], op=ALU.mult)
                nc.vector.tensor_tensor(h_n[:, 128:256], s_rz[:, 256:384],
                                        h_n[:, 128:256], op=ALU.mult)
                nc.vector.tensor_add(h_n[:], h_n[:], i_n[:])
                nc.scalar.activation(h_n[:], h_n[:], AF.Tanh)
                # d = (h_prev - n) * z ; h_new = n + d   (d reuses i_n)
                nc.vector.tensor_sub(i_n[:], h_prev[:], h_n[:])
                nc.vector.tensor_tensor(i_n[:, 0:128], s_rz[:, 128:256],
                                        i_n[:, 0:128], op=ALU.mult)
                nc.vector.tensor_tensor(i_n[:, 128:256], s_rz[:, 384:512],
                                        i_n[:, 128:256], op=ALU.mult)
                h_new = hbm[(t + 1) % 2]
                nc.vector.tensor_add(h_new[:], h_n[:], i_n[:])

                # ---- transpose h_new, AllGather hidden ----
                tr_ps = ps_tr_pool.tile([128, 512], F32, name="tr_ps", tag="tr")
                nc.tensor.matmul(tr_ps[:, 0:32], lhsT=h_new[:, 0:128], rhs=id32,
                                 is_transpose=True, start=True, stop=False)
                nc.tensor.matmul(tr_ps[:, 32:64], lhsT=h_new[:, 128:256], rhs=id32,
                                 is_transpose=True, start=False, stop=True)
                ag1_sb = tpool.tile([128, 64], F32, name="ag1_sb", tag="ag1_sb")
                nc.vector.tensor_copy(ag1_sb[:], tr_ps[:, 0:64])
                ag1_in = dpool.tile([128, 64], F32, name="ag1_in", tag="ag1_in")
                nc.gpsimd.dma_start(ag1_in[:], ag1_sb[:])
                ag1_out = dpool.tile([128 * NC, 64], F32, name="ag1_out",
                                     addr_space="Shared", tag="ag1_out")
                nc.gpsimd.collective_compute(
                    "AllGather", ALU.bypass, replica_groups=rg,
                    ins=[ag1_in.opt()], outs=[ag1_out.opt()])
                nc.gpsimd.dma_start(
                    h_cur[:].rearrange("p (c q) -> p c q", c=8),
                    ag1_out[:].rearrange("(c p) q -> p c q", p=128))
                # bf16 copy of the gathered hidden for the projection
                hTb = gpool.tile([128, 8 * 64], BF16, name="hTb", tag="hTb")
                nc.vector.tensor_copy(hTb[:], h_cur[:])

                # ---- output projection (bf16) ----
                pj = [ps_proj_pool.tile([128, 512], F32, name=f"pj{g}", tag=f"pj{g}")
                      for g in range(GROUPS)]

                def lh_of(c):
                    if c < 8:
                        return hTb[:, c * 64 : c * 64 + 32]
                    return hTb[:, (c - 8) * 64 + 32 : (c - 8) * 64 + 64]

                for c in range(KC):
                    for ch in range(NCH):
                        g, j = divmod(ch, 4)
                        nc.tensor.matmul(
                            pj[g][32 * j : 32 * (j + 1), 0:CH], lhsT=lh_of(c),
                            rhs=wres[:, (c * NCH + ch) * CH : (c * NCH + ch + 1) * CH],
                            start=(c == 0), stop=(c == KC - 1),
                            skip_group_check=True, tile_position=(0, 32 * j))

                # ---- logits epilogue: bias, top-8 per partition row ----
                logits = lpool.tile([128, GROUPS * CH], F32, name="logits", tag="logits")
                for g in range(GROUPS):
                    nc.vector.tensor_add(logits[:, g * CH : (g + 1) * CH],
                                         pj[g][:, 0:CH],
                                         bout[:, g * CH : (g + 1) * CH])
                mx8 = tpool.tile([128, 8], F32, name="mx8", tag="mx8")
                ix8 = tpool.tile([128, 8], U32, name="ix8", tag="ix8")
                nc.vector.max(out=mx8[:], in_=logits[:])
                nc.vector.max_index(out=ix8[:], in_max=mx8[:], in_values=logits[:])
                # local vocab index = offs_l(j) + idx + (idx >= 500) * 1500
                ixf = tpool.tile([128, 8], F32, name="ixf", tag="ixf")
                nc.vector.tensor_copy(ixf[:], ix8[:])
                gmask = tpool.tile([128, 8], F32, name="gmask", tag="gmask")
                nc.vector.tensor_scalar(gmask[:], ixf[:], float(CH), 1500.0,
                                        op0=ALU.is_ge, op1=ALU.mult)
                nc.vector.tensor_add(ixf[:], ixf[:], gmask[:])
                nc.vector.tensor_scalar(ixf[:], ixf[:], offsl[:, 0:1], None,
                                        op0=ALU.add)

                # ---- merge: top-4 of the 4 stacks' top-4s per batch row ----
                cand16 = tpool.tile([B, 16], F32, name="cand16", tag="cand16")
                candi16 = tpool.tile([B, 16], F32, name="candi16", tag="candi16")
                for j in range(4):
                    nc.vector.tensor_copy(cand16[:, 4 * j : 4 * j + 4],
                                          mx8[32 * j : 32 * (j + 1), 0:4])
                    nc.vector.tensor_copy(candi16[:, 4 * j : 4 * j + 4],
                                          ixf[32 * j : 32 * (j + 1), 0:4])
                m16 = tpool.tile([B, 8], F32, name="m16", tag="m16")
                nc.vector.max(out=m16[:], in_=cand16[:])
                iloc4 = tpool.tile([B, NREF], F32, name="iloc4", tag="iloc4")
                for r in range(NREF):
                    mskr = tpool.tile([B, 16], U32, name="mskr", tag="mskr")
                    nc.vector.tensor_scalar(mskr[:], cand16[:], m16[:, r : r + 1],
                                            None, op0=ALU.is_equal)
                    iselr = tpool.tile([B, 16], F32, name="iselr", tag="iselr")
                    nc.vector.tensor_copy(iselr[:], bigt[:])
                    nc.vector.copy_predicated(iselr[:], mskr[:], candi16[:])
                    nc.vector.tensor_reduce(iloc4[:, r : r + 1], iselr[:],
                                            axis=AX.X, op=ALU.min)

                # ---- local sum-exp (vs approx max) — off critical path ----
                mneg_l = tpool.tile([128, 1], F32, name="mneg_l", tag="mneg_l")
                nc.vector.tensor_scalar_mul(mneg_l[0:B, :], m16[:, 0:1], -1.0)
                nc.vector.tensor_copy(mneg_l[B : 2 * B, :], mneg_l[0:B, :])
                nc.vector.tensor_copy(mneg_l[2 * B :, :], mneg_l[0 : 2 * B, :])
                sparts = tpool.tile([128, 2], F32, name="sparts", tag="sparts")
                for g in range(GROUPS):
                    e_ps = ps_exp_pool.tile([128, 512], F32, name="e_ps", tag="exp")
                    nc.scalar.activation(e_ps[:, 0:CH], logits[:, g * CH : (g + 1) * CH],
                                         AF.Exp, bias=mneg_l[:, 0:1],
                                         accum_out=sparts[:, g : g + 1])
                s128 = tpool.tile([128, 1], F32, name="s128", tag="s128")
                nc.vector.tensor_add(s128[:], sparts[:, 0:1], sparts[:, 1:2])
                scand = tpool.tile([B, 4], F32, name="scand", tag="scand")
                for j in range(4):
                    nc.vector.tensor_copy(scand[:, j : j + 1],
                                          s128[32 * j : 32 * (j + 1), :])
                s_loc = tpool.tile([B, 1], F32, name="s_loc", tag="s_loc")
                nc.vector.reduce_sum(s_loc[:], scand[:], axis=AX.X)

                # ---- fp32 refinement of the 4 candidates ----
                i128f = rpool.tile([128, 1], F32, name="i128f", tag="i128f")
                for r in range(NREF):
                    nc.vector.tensor_copy(i128f[32 * r : 32 * (r + 1), :],
                                          iloc4[:, r : r + 1])
                i128 = rpool.tile([128, 1], U32, name="i128", tag="i128")
                nc.vector.tensor_copy(i128[:], i128f[:])
                bcand = rpool.tile([128, 1], F32, name="bcand", tag="bcand")
                nc.gpsimd.indirect_dma_start(
                    out=bcand[:], out_offset=None, in_=boutf_t.ap(),
                    in_offset=bass.IndirectOffsetOnAxis(ap=i128[:, 0:1], axis=0))
                ref_ps = ps_ref_pool.tile([128, 32], F32, name="ref_ps", tag="ref")
                for quarter in range(4):
                    wcand = rpool.tile([128, 512], F32, name="wcand", tag="wcand")
                    nc.gpsimd.indirect_dma_start(
                        out=wcand[:], out_offset=None,
                        in_=wsl_t[quarter].ap(),
                        in_offset=bass.IndirectOffsetOnAxis(ap=i128[:, 0:1], axis=0))
                    wcT = rpool.tile([128, 512], F32, name="wcT", tag="wcT")
                    trw_ps = ps_tr_pool.tile([128, 512], F32, name="trw_ps", tag="tr")
                    for mm in range(4):
                        nc.tensor.matmul(
                            trw_ps[:, mm * 128 : (mm + 1) * 128],
                            lhsT=wcand[:, mm * 128 : (mm + 1) * 128],
                            rhs=ident[:], is_transpose=True,
                            start=(mm == 0), stop=(mm == 3))
                    nc.vector.tensor_copy(wcT[:], trw_ps[:])
                    for mm in range(4):
                        m = quarter * 4 + mm
                        dirn, cm = divmod(m, 8)
                        nc.tensor.matmul(
                            ref_ps[:], lhsT=wcT[:, mm * 128 : (mm + 1) * 128],
                            rhs=h_cur[:, cm * 64 + dirn * 32 : cm * 64 + dirn * 32 + 32],
                            start=(m == 0), stop=(m == KC - 1))
                # diag extract: refined[p] = ref_ps[p, p % 32] + b_out[cand]
                refd = rpool.tile([128, 32], F32, name="refd", tag="refd")
                nc.vector.tensor_tensor(refd[:], ref_ps[:], onehot[:], op=ALU.mult)
                refv = rpool.tile([128, 1], F32, name="refv", tag="refv")
                nc.vector.reduce_sum(refv[:], refd[:], axis=AX.X)
                nc.vector.tensor_add(refv[:], refv[:], bcand[:])
                refc = tpool.tile([B, NREF], F32, name="refc", tag="refc")
                iglob4 = tpool.tile([B, NREF], F32, name="iglob4", tag="iglob4")
                for r in range(NREF):
                    nc.vector.tensor_copy(refc[:, r : r + 1],
                                          refv[32 * r : 32 * (r + 1), :])
                nc.vector.tensor_scalar(iglob4[:], iloc4[:], v0t[:, 0:1], None,
                                        op0=ALU.add)
                m_loc = tpool.tile([B, 1], F32, name="m_loc", tag="m_loc")
                nc.vector.reduce_max(m_loc[:], refc[:], axis=AX.X)
                msk = tpool.tile([B, NREF], U32, name="msk", tag="msk")
                nc.vector.tensor_scalar(msk[:], refc[:], m_loc[:], None, op0=ALU.is_equal)
                isel = tpool.tile([B, NREF], F32, name="isel", tag="isel")
                nc.vector.tensor_copy(isel[:], bigt[:, 0:NREF])
                nc.vector.copy_predicated(isel[:], msk[:], iglob4[:])
                i_loc = tpool.tile([B, 1], F32, name="i_loc", tag="i_loc")
                nc.vector.tensor_reduce(i_loc[:], isel[:], axis=AX.X, op=ALU.min)

                # ---- AG2: (m_ref, idx, m_approx, s) from all cores ----
                ag2_sb = tpool.tile([B, 4], F32, name="ag2_sb", tag="ag2_sb")
                nc.vector.tensor_copy(ag2_sb[:, 0:1], m_loc[:])
                nc.vector.tensor_copy(ag2_sb[:, 1:2], i_loc[:])
                nc.vector.tensor_copy(ag2_sb[:, 2:3], m16[:, 0:1])
                nc.vector.tensor_copy(ag2_sb[:, 3:4], s_loc[:])
                ag2_in = dpool.tile([B, 4], F32, name="ag2_in", tag="ag2_in")
                nc.gpsimd.dma_start(ag2_in[:], ag2_sb[:])
                ag2_out = dpool.tile([B * NC, 4], F32, name="ag2_out",
                                     addr_space="Shared", tag="ag2_out")
                nc.gpsimd.collective_compute(
                    "AllGather", ALU.bypass, replica_groups=rg,
                    ins=[ag2_in.opt()], outs=[ag2_out.opt()])
                unp2 = tpool.tile([B, 32], F32, name="unp2", tag="unp2")
                nc.gpsimd.dma_start(
                    unp2[:].rearrange("b (r c) -> b r c", r=NC),
                    ag2_out[:].rearrange("(r b) c -> b r c", b=B))
                vals = bass.AP(unp2.tensor, unp2[:].offset,
                               [unp2[:].ap[0], [4, 8]])
                idxs = bass.AP(unp2.tensor, unp2[:].offset + 1,
                               [unp2[:].ap[0], [4, 8]])
                mtils = bass.AP(unp2.tensor, unp2[:].offset + 2,
                                [unp2[:].ap[0], [4, 8]])
                svals = bass.AP(unp2.tensor, unp2[:].offset + 3,
                                [unp2[:].ap[0], [4, 8]])
                m_glob = tpool.tile([B, 1], F32, name="m_glob", tag="m_glob")
                nc.vector.reduce_max(m_glob[:], vals, axis=AX.X)
                msk2 = tpool.tile([B, 8], U32, name="msk2", tag="msk2")
                nc.vector.tensor_scalar(msk2[:], vals, m_glob[:], None, op0=ALU.is_equal)
                isel2 = tpool.tile([B, 8], F32, name="isel2", tag="isel2")
                nc.vector.tensor_copy(isel2[:], bigt[:, 0:8])
                nc.vector.copy_predicated(isel2[:], msk2[:], idxs)
                i_glob = tpool.tile([B, 1], F32, name="i_glob", tag="i_glob")
                nc.vector.tensor_reduce(i_glob[:], isel2[:], axis=AX.X, op=ALU.min)
                # logZ = max(m_approx) + ln(sum s_k exp(m_approx_k - max))
                mz = tpool.tile([B, 1], F32, name="mz", tag="mz")
                nc.vector.reduce_max(mz[:], mtils, axis=AX.X)
                dmx = tpool.tile([B, 8], F32, name="dmx", tag="dmx")
                nc.vector.tensor_scalar(dmx[:], mtils, mz[:], None, op0=ALU.subtract)
                nc.scalar.activation(dmx[:], dmx[:], AF.Exp)
                nc.vector.tensor_tensor(dmx[:], dmx[:], svals, op=ALU.mult)
                s_glob = tpool.tile([B, 1], F32, name="s_glob", tag="s_glob")
                nc.vector.reduce_sum(s_glob[:], dmx[:], axis=AX.X)
                lns = tpool.tile([B, 1], F32, name="lns", tag="lns")
                nc.scalar.activation(lns[:], s_glob[:], AF.Ln)
                logz = tpool.tile([128, 1], F32, name="logz", tag="logz", bufs=2)
                nc.vector.tensor_add(logz[0:B, :], lns[:], mz[:])
                nc.vector.tensor_copy(logz[B : 2 * B, :], logz[0:B, :])
                nc.vector.tensor_copy(logz[2 * B :, :], logz[0 : 2 * B, :])

                # ---- prefetch for t+1: gh matmuls, token embed, transpose ----
                if t + 1 < T:
                    rz_ps_next = ps_rz_pool.tile([B, 512], F32, name="rz_ps", tag="rz")
                    inhn_ps_next = ps_n_pool.tile([B, 512], F32, name="inhn_ps", tag="inhn")
                    emit_gh(t + 1, rz_ps_next, inhn_ps_next[:, 0:256])
                    tok = tpool.tile([B, 1], U32, name="tok", tag="tok")
                    nc.vector.tensor_copy(tok[:], i_glob[:])
                    x_sb = tpool.tile([B, H], F32, name="x_sb", tag="x_sb", bufs=1)
                    nc.gpsimd.indirect_dma_start(
                        out=x_sb[:], out_offset=None, in_=emb_t.ap(),
                        in_offset=bass.IndirectOffsetOnAxis(ap=tok[:, 0:1], axis=0))
                    xtr_ps = ps_tr_pool.tile([128, 512], F32, name="xtr_ps", tag="tr")
                    for c in range(8):
                        nc.tensor.matmul(xtr_ps[:, c * 32 : (c + 1) * 32],
                                         lhsT=x_sb[:, c * 128 : (c + 1) * 128],
                                         rhs=id32, is_transpose=True,
                                         start=(c == 0), stop=(c == 7))
                    nc.vector.tensor_copy(xT[(t + 1) % 2][:], xtr_ps[:, 0:256])

                # ---- logp = logits - logZ; write out ----
                nc.gpsimd.tensor_scalar(logits[:], logits[:], logz[:, 0:1], None,
                                        op0=ALU.subtract)
                nc.gpsimd.dma_start(logp_t.ap()[t * 128 : (t + 1) * 128, :], logits[:])

    nc.compile()
    return nc


def prep_inputs(inputs, hidden, emb, w_ih_f, w_hh_f, b_ih_f, b_hh_f,
                w_ih_b, w_hh_b, b_ih_b, b_hh_b, w_out, b_out):
    """Build the per-core input maps (all numpy, host-side sharding)."""
    BF16_NP = mybir.dt.np(mybir.dt.bfloat16)
    emb = np.ascontiguousarray(np.asarray(emb), dtype=np.float32)
    w_out = np.asarray(w_out)
    tok0 = np.asarray(inputs)[:, 0].astype(np.int64)
    x0 = emb[tok0]                                              # (B, H)
    hidden = np.asarray(hidden)
    h_f0, h_b0 = hidden[0], hidden[1]                           # (B, H)

    x0t = np.ascontiguousarray(x0.T).reshape(8, 128, B).transpose(1, 0, 2) \
        .reshape(128, 8 * B).astype(np.float32)
    ht0 = np.empty((128, 8, 64), dtype=np.float32)
    ht0[:, :, 0:32] = np.ascontiguousarray(h_f0.T).reshape(8, 128, B).transpose(1, 0, 2)
    ht0[:, :, 32:64] = np.ascontiguousarray(h_b0.T).reshape(8, 128, B).transpose(1, 0, 2)
    ht0 = ht0.reshape(128, 8 * 64)

    wihf, whhf = np.asarray(w_ih_f), np.asarray(w_hh_f)
    wihb, whhb = np.asarray(w_ih_b), np.asarray(w_hh_b)
    bihf, bhhf = np.asarray(b_ih_f), np.asarray(b_hh_f)
    bihb, bhhb = np.asarray(b_ih_b), np.asarray(b_hh_b)

    onehot = np.zeros((128, B), dtype=np.float32)
    for p in range(128):
        onehot[p, p % 32] = 1.0

    in_maps = []
    for k in range(NC):
        v0 = Vs * k
        sl = [slice(g * H + Hs * k, g * H + Hs * (k + 1)) for g in range(3)]

        w_slice = np.ascontiguousarray(w_out[v0 : v0 + Vs, :], dtype=np.float32)
        w_oT = np.ascontiguousarray(w_slice.T)                  # (2048, Vs)
        wres = w_oT.reshape(KC, 128, Vs).transpose(1, 0, 2) \
            .reshape(128, KC * Vs).astype(BF16_NP).copy()

        def gcat(wf, wb):
            cols = [wf[sl[0]].T, wf[sl[1]].T, wb[sl[0]].T, wb[sl[1]].T,
                    wf[sl[2]].T, wb[sl[2]].T]
            cat = np.concatenate(cols, axis=1)                   # (1024, 768)
            return cat.reshape(8, 128, 768).transpose(1, 0, 2) \
                .reshape(128, 8 * 768).astype(np.float32).copy()

        def bcast(v):
            return np.broadcast_to(v.astype(np.float32), (B, v.size)).copy()

        brz = bcast(np.concatenate([bihf[sl[0]] + bhhf[sl[0]],
                                    bihf[sl[1]] + bhhf[sl[1]],
                                    bihb[sl[0]] + bhhb[sl[0]],
                                    bihb[sl[1]] + bhhb[sl[1]]]))
        b_in_ = bcast(np.concatenate([bihf[sl[2]], bihb[sl[2]]]))
        b_hn_ = bcast(np.concatenate([bhhf[sl[2]], bhhb[sl[2]]]))

        bo_slice = np.asarray(b_out)[v0 : v0 + Vs].astype(np.float32)
        bo = bo_slice.reshape(GROUPS, 4, CH)
        boutt = np.empty((128, GROUPS * CH), dtype=BF16_NP)
        for g in range(GROUPS):
            for j in range(4):
                boutt[32 * j : 32 * (j + 1), g * CH : (g + 1) * CH] = bo[g, j]

        # per-partition (32j+b) local vocab base: j*500
        of = np.empty((128, 1), dtype=np.float32)
        for j in range(4):
            of[32 * j : 32 * (j + 1), 0] = j * CH

        hbm0 = np.concatenate([h_f0[:, Hs * k : Hs * (k + 1)],
                               h_b0[:, Hs * k : Hs * (k + 1)]], axis=1) \
            .astype(np.float32).copy()

        in_maps.append({
            "emb_t": emb, "wres_t": wres,
            **{f"wsl{q}_t": np.ascontiguousarray(w_slice[:, q * 512 : (q + 1) * 512])
               for q in range(4)},
            "boutf_t": bo_slice.reshape(Vs, 1).copy(),
            "wih_t": gcat(wihf, wihb), "whh_t": gcat(whhf, whhb),
            "brz_t": brz, "bin_t": b_in_, "bhn_t": b_hn_,
            "bout_t": boutt, "offsl_t": of,
            "v0_t": np.full((B, 1), float(v0), dtype=np.float32),
            "onehot_t": onehot,
            "ht0_t": ht0, "hbm0_t": hbm0, "x0t_t": x0t,
        })
    return in_maps


_CACHE = {}


def _get_program(T, **kw):
    key = (T, tuple(sorted(kw.items())))
    if key not in _CACHE:
        _CACHE[key] = build_program(T, **kw)
    return _CACHE[key]


def run(T, in_maps, trace=False):
    nc = _get_program(T)
    res = bass_utils.run_bass_kernel_spmd(
        nc, in_maps, core_ids=list(range(NC)), trace=trace)
    outs = []
    for k in range(NC):
        arr = res.results[k]["logp_t"].reshape(T, 4, B, GROUPS, CH)
        outs.append(arr.transpose(2, 0, 3, 1, 4).reshape(B, T, Vs))
    return np.concatenate(outs, axis=2), res


def kernel(inputs, hidden, emb, w_ih_f, w_hh_f, b_ih_f, b_hh_f,
           w_ih_b, w_hh_b, b_ih_b, b_hh_b, w_out, b_out, output_len):
    T = int(output_len)
    in_maps = prep_inputs(inputs, hidden, emb, w_ih_f, w_hh_f, b_ih_f, b_hh_f,
                          w_ih_b, w_hh_b, b_ih_b, b_hh_b, w_out, b_out)
    out, _ = run(T, in_maps)
    return out
